# revision 1
# baseline (speedup 1.0000x reference)
"""2D orthonormal DCT-II over [32,64,224,224], data-parallel on 8 TRN2 cores.

Math per image X [224,224]:  Y = Dh @ X @ Dw.T  (Dh = Dw = 224-pt DCT-II).
Implemented as two PE matmul stages with the *data* as the stationary
operand, which absorbs both transposes:
  stage 1:  T[w,k] = sum_h X[h,w] * DhT[h,k]      (T = (Dh @ X)^T)
  stage 2:  Y[k,l] = sum_w T[w,k] * DwT[w,l]
Matmuls run in float32r (rounded fp32, ~1.6e-4 rel err, 1 cyc/row on the
PE vs 4 for plain fp32). Contraction dim 224 is split 128+96 across two
accumulating matmuls; output partitions 224 likewise split 128+96.
"""
import numpy as np
import concourse.bacc as bacc
import concourse.mybir as mybir
import concourse.tile as tile
from concourse.bass_utils import run_bass_kernel_spmd

B, C, H, W = 32, 64, 224, 224
N_CORES = 8
IMGS = B * C // N_CORES  # images per core
G = 8                    # images per DMA group
P0, P1 = 128, H - 128    # partition split of the 224 dim

f32 = mybir.dt.float32
f32r = mybir.dt.float32r

_cache = {}


def _dct2_matrix(n: int) -> np.ndarray:
    k = np.arange(n)[:, None].astype(np.float64)
    m = np.arange(n)[None, :].astype(np.float64)
    d = np.cos(np.pi * (2.0 * m + 1.0) * k / (2.0 * n))
    scale = np.full((n, 1), np.sqrt(2.0 / n))
    scale[0, 0] = np.sqrt(1.0 / n)
    return (scale * d).astype(np.float32)


def _build():
    nc = bacc.Bacc("TRN2", target_bir_lowering=False, debug=False)
    x_d = nc.dram_tensor("x", [IMGS, H, W], f32, kind="ExternalInput").ap()
    dht_d = nc.dram_tensor("dht", [H, H], f32, kind="ExternalInput").ap()
    dwt_d = nc.dram_tensor("dwt", [W, W], f32, kind="ExternalInput").ap()
    y_d = nc.dram_tensor("y", [IMGS, H, W], f32, kind="ExternalOutput").ap()

    with tile.TileContext(nc) as tc:
        with (
            tc.tile_pool(name="consts", bufs=1) as cpool,
            tc.tile_pool(name="xin", bufs=2) as xpool,
            tc.tile_pool(name="xr", bufs=2) as xrpool,
            tc.tile_pool(name="tmid", bufs=4) as tpool,
            tc.tile_pool(name="yout", bufs=2) as ypool,
            tc.tile_pool(name="pst", bufs=2, space="PSUM") as pst,
            tc.tile_pool(name="psy", bufs=2, space="PSUM") as psy,
        ):
            # DCT matrices: stage fp32, round once to f32r
            dht_s0 = cpool.tile([P0, H], f32)
            dht_s1 = cpool.tile([P1, H], f32)
            dwt_s0 = cpool.tile([P0, W], f32)
            dwt_s1 = cpool.tile([P1, W], f32)
            nc.sync.dma_start(dht_s0, dht_d[0:P0, :])
            nc.sync.dma_start(dht_s1, dht_d[P0:H, :])
            nc.sync.dma_start(dwt_s0, dwt_d[0:P0, :])
            nc.sync.dma_start(dwt_s1, dwt_d[P0:W, :])
            dht0 = cpool.tile([P0, H], f32r)
            dht1 = cpool.tile([P1, H], f32r)
            dwt0 = cpool.tile([P0, W], f32r)
            dwt1 = cpool.tile([P1, W], f32r)
            nc.vector.tensor_copy(dht0, dht_s0)
            nc.vector.tensor_copy(dht1, dht_s1)
            nc.vector.tensor_copy(dwt0, dwt_s0)
            nc.vector.tensor_copy(dwt1, dwt_s1)

            for g in range(IMGS // G):
                sl = slice(g * G, (g + 1) * G)
                x0 = xpool.tile([P0, G, W], f32, name="x0", tag="x0")
                x1 = xpool.tile([P1, G, W], f32, name="x1", tag="x1")
                nc.sync.dma_start(x0, x_d[sl, 0:P0, :].transpose([1, 0, 2]))
                nc.sync.dma_start(x1, x_d[sl, P0:H, :].transpose([1, 0, 2]))
                x0r = xrpool.tile([P0, G, W], f32r, name="x0r", tag="x0r")
                x1r = xrpool.tile([P1, G, W], f32r, name="x1r", tag="x1r")
                nc.vector.tensor_copy(x0r, x0)
                nc.vector.tensor_copy(x1r, x1)
                ys0 = ypool.tile([P0, G, W], f32, name="ys0", tag="ys0")
                ys1 = ypool.tile([P1, G, W], f32, name="ys1", tag="ys1")

                for j in range(G):
                    # stage 1: T = (Dh @ X)^T, two partition chunks
                    t0p = pst.tile([P0, H], f32, name="t0p", tag="t0p")
                    t1p = pst.tile([P1, H], f32, name="t1p", tag="t1p")
                    nc.tensor.matmul(t0p, x0r[:, j, 0:P0], dht0,
                                     start=True, stop=False)
                    nc.tensor.matmul(t0p, x1r[:, j, 0:P0], dht1,
                                     start=False, stop=True)
                    nc.tensor.matmul(t1p, x0r[:, j, P0:W], dht0,
                                     start=True, stop=False)
                    nc.tensor.matmul(t1p, x1r[:, j, P0:W], dht1,
                                     start=False, stop=True)
                    t0r = tpool.tile([P0, H], f32r, name="t0r", tag="t0r")
                    t1r = tpool.tile([P1, H], f32r, name="t1r", tag="t1r")
                    nc.vector.tensor_copy(t0r, t0p)
                    nc.vector.tensor_copy(t1r, t1p)
                    # stage 2: Y = T^T @ DwT, two partition chunks
                    y0p = psy.tile([P0, W], f32, name="y0p", tag="y0p")
                    y1p = psy.tile([P1, W], f32, name="y1p", tag="y1p")
                    nc.tensor.matmul(y0p, t0r[:, 0:P0], dwt0,
                                     start=True, stop=False)
                    nc.tensor.matmul(y0p, t1r[:, 0:P0], dwt1,
                                     start=False, stop=True)
                    nc.tensor.matmul(y1p, t0r[:, P0:H], dwt0,
                                     start=True, stop=False)
                    nc.tensor.matmul(y1p, t1r[:, P0:H], dwt1,
                                     start=False, stop=True)
                    nc.scalar.copy(ys0[:, j, :], y0p)
                    nc.scalar.copy(ys1[:, j, :], y1p)

                nc.sync.dma_start(y_d[sl, 0:P0, :].transpose([1, 0, 2]), ys0)
                nc.sync.dma_start(y_d[sl, P0:H, :].transpose([1, 0, 2]), ys1)

    nc.compile()
    return nc


def _run(x: np.ndarray, trace: bool = False):
    """x: [B, C, H, W] fp32. Returns (y, BassKernelResults)."""
    if "nc" not in _cache:
        _cache["nc"] = _build()
    nc = _cache["nc"]
    d = _dct2_matrix(H)
    dt_ = np.ascontiguousarray(d.T)  # DhT[h, k] = Dh[k, h]; Dh == Dw here
    flat = np.ascontiguousarray(x.reshape(B * C, H, W).astype(np.float32))
    in_maps = [
        {"x": flat[i * IMGS:(i + 1) * IMGS], "dht": dt_, "dwt": dt_}
        for i in range(N_CORES)
    ]
    res = run_bass_kernel_spmd(nc, in_maps, core_ids=list(range(N_CORES)),
                               trace=trace)
    y = np.concatenate([r["y"] for r in res.results], axis=0)
    return y.reshape(B, C, H, W), res


def kernel(x: np.ndarray) -> np.ndarray:
    y, _ = _run(np.asarray(x))
    return y


# revision 3
# speedup vs baseline: 1.0101x; 1.0101x over previous
"""2D orthonormal DCT-II over [32,64,224,224], data-parallel on 8 TRN2 cores.

Math per image X [224,224]:  Y = Dh @ X @ Dw.T  (Dh = Dw = 224-pt DCT-II).
Implemented as two PE matmul stages with the *data* as the stationary
operand, which absorbs both transposes:
  stage 1:  T[w,k] = sum_h X[h,w] * DhT[h,k]      (T = (Dh @ X)^T)
  stage 2:  Y[k,l] = sum_w T[w,k] * DwT[w,l]
Matmuls run in float32r (rounded fp32, ~1.6e-4 rel err, 1 cyc/row on the
PE vs 4 for plain fp32). Contraction dim 224 is split 128+96 across two
accumulating matmuls; output partitions 224 likewise split 128+96.
"""
import numpy as np
import concourse.bacc as bacc
import concourse.mybir as mybir
import concourse.tile as tile
from concourse.bass_utils import run_bass_kernel_spmd

B, C, H, W = 32, 64, 224, 224
N_CORES = 8
IMGS = B * C // N_CORES  # images per core
G = 8                    # images per DMA group
P0, P1 = 128, H - 128    # partition split of the 224 dim

f32 = mybir.dt.float32
f32r = mybir.dt.float32r

_cache = {}


def _dct2_matrix(n: int) -> np.ndarray:
    k = np.arange(n)[:, None].astype(np.float64)
    m = np.arange(n)[None, :].astype(np.float64)
    d = np.cos(np.pi * (2.0 * m + 1.0) * k / (2.0 * n))
    scale = np.full((n, 1), np.sqrt(2.0 / n))
    scale[0, 0] = np.sqrt(1.0 / n)
    return (scale * d).astype(np.float32)


def _build():
    nc = bacc.Bacc("TRN2", target_bir_lowering=False, debug=False)
    x_d = nc.dram_tensor("x", [IMGS, H, W], f32, kind="ExternalInput").ap()
    dht_d = nc.dram_tensor("dht", [H, H], f32, kind="ExternalInput").ap()
    dwt_d = nc.dram_tensor("dwt", [W, W], f32, kind="ExternalInput").ap()
    y_d = nc.dram_tensor("y", [IMGS, H, W], f32, kind="ExternalOutput").ap()

    with tile.TileContext(nc) as tc:
        with (
            tc.tile_pool(name="consts", bufs=1) as cpool,
            tc.tile_pool(name="xin", bufs=2) as xpool,
            tc.tile_pool(name="xr", bufs=2) as xrpool,
            tc.tile_pool(name="tmid", bufs=4) as tpool,
            tc.tile_pool(name="yout", bufs=2) as ypool,
            tc.tile_pool(name="pst", bufs=2, space="PSUM") as pst,
            tc.tile_pool(name="psy", bufs=2, space="PSUM") as psy,
        ):
            # DCT matrices: stage fp32, round once to f32r
            dht_s0 = cpool.tile([P0, H], f32)
            dht_s1 = cpool.tile([P1, H], f32)
            dwt_s0 = cpool.tile([P0, W], f32)
            dwt_s1 = cpool.tile([P1, W], f32)
            nc.sync.dma_start(dht_s0, dht_d[0:P0, :])
            nc.sync.dma_start(dht_s1, dht_d[P0:H, :])
            nc.sync.dma_start(dwt_s0, dwt_d[0:P0, :])
            nc.sync.dma_start(dwt_s1, dwt_d[P0:W, :])
            dht0 = cpool.tile([P0, H], f32r)
            dht1 = cpool.tile([P1, H], f32r)
            dwt0 = cpool.tile([P0, W], f32r)
            dwt1 = cpool.tile([P1, W], f32r)
            nc.vector.tensor_copy(dht0, dht_s0)
            nc.vector.tensor_copy(dht1, dht_s1)
            nc.vector.tensor_copy(dwt0, dwt_s0)
            nc.vector.tensor_copy(dwt1, dwt_s1)

            # PE warmup: ~10us of dense junk matmuls to trip the HAM
            # clock-gate to K=8/8 (2.4 GHz) before the real work starts.
            bf16 = mybir.dt.bfloat16
            junk_w = cpool.tile([P0, P0], bf16)
            junk_m = cpool.tile([P0, 512], bf16)
            nc.gpsimd.memset(junk_w, 0)
            nc.gpsimd.memset(junk_m, 0)
            for r in range(24):
                wp = pst.tile([P0, 512], f32, name=f"warm{r}", tag="t0p")
                nc.tensor.matmul(wp, junk_w, junk_m, start=True, stop=True)

            for g in range(IMGS // G):
                sl = slice(g * G, (g + 1) * G)
                x0 = xpool.tile([P0, G, W], f32, name="x0", tag="x0")
                x1 = xpool.tile([P1, G, W], f32, name="x1", tag="x1")
                nc.sync.dma_start(x0, x_d[sl, 0:P0, :].transpose([1, 0, 2]))
                nc.sync.dma_start(x1, x_d[sl, P0:H, :].transpose([1, 0, 2]))
                x0r = xrpool.tile([P0, G, W], f32r, name="x0r", tag="x0r")
                x1r = xrpool.tile([P1, G, W], f32r, name="x1r", tag="x1r")
                nc.gpsimd.tensor_copy(x0r, x0)
                nc.gpsimd.tensor_copy(x1r, x1)
                ys0 = ypool.tile([P0, G, W], f32, name="ys0", tag="ys0")
                ys1 = ypool.tile([P1, G, W], f32, name="ys1", tag="ys1")

                for j in range(G):
                    # stage 1: T = (Dh @ X)^T, two partition chunks
                    t0p = pst.tile([P0, H], f32, name="t0p", tag="t0p")
                    t1p = pst.tile([P1, H], f32, name="t1p", tag="t1p")
                    nc.tensor.matmul(t0p, x0r[:, j, 0:P0], dht0,
                                     start=True, stop=False)
                    nc.tensor.matmul(t0p, x1r[:, j, 0:P0], dht1,
                                     start=False, stop=True)
                    nc.tensor.matmul(t1p, x0r[:, j, P0:W], dht0,
                                     start=True, stop=False)
                    nc.tensor.matmul(t1p, x1r[:, j, P0:W], dht1,
                                     start=False, stop=True)
                    t0r = tpool.tile([P0, H], f32r, name="t0r", tag="t0r")
                    t1r = tpool.tile([P1, H], f32r, name="t1r", tag="t1r")
                    nc.vector.tensor_copy(t0r, t0p)
                    nc.scalar.copy(t1r, t1p)
                    # stage 2: Y = T^T @ DwT, two partition chunks
                    y0p = psy.tile([P0, W], f32, name="y0p", tag="y0p")
                    y1p = psy.tile([P1, W], f32, name="y1p", tag="y1p")
                    nc.tensor.matmul(y0p, t0r[:, 0:P0], dwt0,
                                     start=True, stop=False)
                    nc.tensor.matmul(y0p, t1r[:, 0:P0], dwt1,
                                     start=False, stop=True)
                    nc.tensor.matmul(y1p, t0r[:, P0:H], dwt0,
                                     start=True, stop=False)
                    nc.tensor.matmul(y1p, t1r[:, P0:H], dwt1,
                                     start=False, stop=True)
                    nc.scalar.copy(ys0[:, j, :], y0p)
                    nc.vector.tensor_copy(ys1[:, j, :], y1p)

                nc.sync.dma_start(y_d[sl, 0:P0, :].transpose([1, 0, 2]), ys0)
                nc.sync.dma_start(y_d[sl, P0:H, :].transpose([1, 0, 2]), ys1)

    nc.compile()
    return nc


def _run(x: np.ndarray, trace: bool = False):
    """x: [B, C, H, W] fp32. Returns (y, BassKernelResults)."""
    if "nc" not in _cache:
        _cache["nc"] = _build()
    nc = _cache["nc"]
    d = _dct2_matrix(H)
    dt_ = np.ascontiguousarray(d.T)  # DhT[h, k] = Dh[k, h]; Dh == Dw here
    flat = np.ascontiguousarray(x.reshape(B * C, H, W).astype(np.float32))
    in_maps = [
        {"x": flat[i * IMGS:(i + 1) * IMGS], "dht": dt_, "dwt": dt_}
        for i in range(N_CORES)
    ]
    res = run_bass_kernel_spmd(nc, in_maps, core_ids=list(range(N_CORES)),
                               trace=trace)
    y = np.concatenate([r["y"] for r in res.results], axis=0)
    return y.reshape(B, C, H, W), res


def kernel(x: np.ndarray) -> np.ndarray:
    y, _ = _run(np.asarray(x))
    return y


# revision 4
# speedup vs baseline: 1.1014x; 1.0904x over previous
"""2D orthonormal DCT-II over [32,64,224,224], data-parallel on 8 TRN2 cores.

Math per image X [224,224]:  Y = Dh @ X @ Dw.T  (Dh = Dw = 224-pt DCT-II).
Implemented as two PE matmul stages with the *data* as the stationary
operand, which absorbs both transposes:
  stage 1:  T[w,k] = sum_h X[h,w] * DhT[h,k]      (T = (Dh @ X)^T)
  stage 2:  Y[k,l] = sum_w T[w,k] * DwT[w,l]
Matmuls run in float32r (rounded fp32, ~1.6e-4 rel err, 1 cyc/row on the
PE vs 4 for plain fp32). Contraction dim 224 is split 128+96 across two
accumulating matmuls; output partitions 224 likewise split 128+96.
"""
import numpy as np
import concourse.bacc as bacc
import concourse.mybir as mybir
import concourse.tile as tile
from concourse.bass_utils import run_bass_kernel_spmd

B, C, H, W = 32, 64, 224, 224
N_CORES = 8
IMGS = B * C // N_CORES  # images per core
G = 8                    # images per DMA group
P0, P1 = 128, H - 128    # partition split of the 224 dim
NS = 320                 # matmul stream width: 224 real + zero pad (HAM duty)

f32 = mybir.dt.float32
f32r = mybir.dt.float32r

_cache = {}


def _dct2_matrix(n: int) -> np.ndarray:
    k = np.arange(n)[:, None].astype(np.float64)
    m = np.arange(n)[None, :].astype(np.float64)
    d = np.cos(np.pi * (2.0 * m + 1.0) * k / (2.0 * n))
    scale = np.full((n, 1), np.sqrt(2.0 / n))
    scale[0, 0] = np.sqrt(1.0 / n)
    return (scale * d).astype(np.float32)


def _build():
    nc = bacc.Bacc("TRN2", target_bir_lowering=False, debug=False)
    x_d = nc.dram_tensor("x", [IMGS, H, W], f32, kind="ExternalInput").ap()
    dht_d = nc.dram_tensor("dht", [H, H], f32, kind="ExternalInput").ap()
    dwt_d = nc.dram_tensor("dwt", [W, W], f32, kind="ExternalInput").ap()
    y_d = nc.dram_tensor("y", [IMGS, H, W], f32, kind="ExternalOutput").ap()

    with tile.TileContext(nc) as tc:
        with (
            tc.tile_pool(name="consts", bufs=1) as cpool,
            tc.tile_pool(name="xin", bufs=2) as xpool,
            tc.tile_pool(name="xr", bufs=2) as xrpool,
            tc.tile_pool(name="tmid", bufs=4) as tpool,
            tc.tile_pool(name="yout", bufs=2) as ypool,
            tc.tile_pool(name="pst", bufs=2, space="PSUM") as pst,
            tc.tile_pool(name="psy", bufs=2, space="PSUM") as psy,
        ):
            # DCT matrices: stage fp32 (zero-padded to NS cols), round to f32r
            dht_s0 = cpool.tile([P0, NS], f32)
            dht_s1 = cpool.tile([P1, NS], f32)
            dwt_s0 = cpool.tile([P0, NS], f32)
            dwt_s1 = cpool.tile([P1, NS], f32)
            for t in (dht_s0, dht_s1, dwt_s0, dwt_s1):
                nc.gpsimd.memset(t, 0)
            nc.sync.dma_start(dht_s0[:, 0:H], dht_d[0:P0, :])
            nc.sync.dma_start(dht_s1[:, 0:H], dht_d[P0:H, :])
            nc.sync.dma_start(dwt_s0[:, 0:W], dwt_d[0:P0, :])
            nc.sync.dma_start(dwt_s1[:, 0:W], dwt_d[P0:W, :])
            dht0 = cpool.tile([P0, NS], f32r)
            dht1 = cpool.tile([P1, NS], f32r)
            dwt0 = cpool.tile([P0, NS], f32r)
            dwt1 = cpool.tile([P1, NS], f32r)
            nc.vector.tensor_copy(dht0, dht_s0)
            nc.vector.tensor_copy(dht1, dht_s1)
            nc.vector.tensor_copy(dwt0, dwt_s0)
            nc.vector.tensor_copy(dwt1, dwt_s1)

            # PE warmup: ~10us of dense junk matmuls to trip the HAM
            # clock-gate to K=8/8 (2.4 GHz) before the real work starts.
            bf16 = mybir.dt.bfloat16
            junk_w = cpool.tile([P0, P0], bf16)
            junk_m = cpool.tile([P0, 512], bf16)
            nc.gpsimd.memset(junk_w, 0)
            nc.gpsimd.memset(junk_m, 0)
            for r in range(40):
                wp = pst.tile([P0, 512], f32, name=f"warm{r}", tag="t0p")
                nc.tensor.matmul(wp, junk_w, junk_m, start=True, stop=True)

            for g in range(IMGS // G):
                sl = slice(g * G, (g + 1) * G)
                x0 = xpool.tile([P0, G, W], f32, name="x0", tag="x0")
                x1 = xpool.tile([P1, G, W], f32, name="x1", tag="x1")
                nc.sync.dma_start(x0, x_d[sl, 0:P0, :].transpose([1, 0, 2]))
                nc.sync.dma_start(x1, x_d[sl, P0:H, :].transpose([1, 0, 2]))
                x0r = xrpool.tile([P0, G, W], f32r, name="x0r", tag="x0r")
                x1r = xrpool.tile([P1, G, W], f32r, name="x1r", tag="x1r")
                nc.gpsimd.tensor_copy(x0r, x0)
                nc.gpsimd.tensor_copy(x1r, x1)
                ys0 = ypool.tile([P0, G, W], f32, name="ys0", tag="ys0")
                ys1 = ypool.tile([P1, G, W], f32, name="ys1", tag="ys1")

                for j in range(G):
                    # stage 1: T = (Dh @ X)^T, two partition chunks
                    t0p = pst.tile([P0, NS], f32, name="t0p", tag="t0p")
                    t1p = pst.tile([P1, NS], f32, name="t1p", tag="t1p")
                    nc.tensor.matmul(t0p, x0r[:, j, 0:P0], dht0,
                                     start=True, stop=False)
                    nc.tensor.matmul(t0p, x1r[:, j, 0:P0], dht1,
                                     start=False, stop=True)
                    nc.tensor.matmul(t1p, x0r[:, j, P0:W], dht0,
                                     start=True, stop=False)
                    nc.tensor.matmul(t1p, x1r[:, j, P0:W], dht1,
                                     start=False, stop=True)
                    t0r = tpool.tile([P0, H], f32r, name="t0r", tag="t0r")
                    t1r = tpool.tile([P1, H], f32r, name="t1r", tag="t1r")
                    nc.vector.tensor_copy(t0r, t0p[:, 0:H])
                    nc.scalar.copy(t1r, t1p[:, 0:H])
                    # stage 2: Y = T^T @ DwT, two partition chunks
                    y0p = psy.tile([P0, NS], f32, name="y0p", tag="y0p")
                    y1p = psy.tile([P1, NS], f32, name="y1p", tag="y1p")
                    nc.tensor.matmul(y0p, t0r[:, 0:P0], dwt0,
                                     start=True, stop=False)
                    nc.tensor.matmul(y0p, t1r[:, 0:P0], dwt1,
                                     start=False, stop=True)
                    nc.tensor.matmul(y1p, t0r[:, P0:H], dwt0,
                                     start=True, stop=False)
                    nc.tensor.matmul(y1p, t1r[:, P0:H], dwt1,
                                     start=False, stop=True)
                    nc.scalar.copy(ys0[:, j, :], y0p[:, 0:W])
                    nc.vector.tensor_copy(ys1[:, j, :], y1p[:, 0:W])

                nc.sync.dma_start(y_d[sl, 0:P0, :].transpose([1, 0, 2]), ys0)
                nc.sync.dma_start(y_d[sl, P0:H, :].transpose([1, 0, 2]), ys1)

    nc.compile()
    return nc


def _run(x: np.ndarray, trace: bool = False):
    """x: [B, C, H, W] fp32. Returns (y, BassKernelResults)."""
    if "nc" not in _cache:
        _cache["nc"] = _build()
    nc = _cache["nc"]
    d = _dct2_matrix(H)
    dt_ = np.ascontiguousarray(d.T)  # DhT[h, k] = Dh[k, h]; Dh == Dw here
    flat = np.ascontiguousarray(x.reshape(B * C, H, W).astype(np.float32))
    in_maps = [
        {"x": flat[i * IMGS:(i + 1) * IMGS], "dht": dt_, "dwt": dt_}
        for i in range(N_CORES)
    ]
    res = run_bass_kernel_spmd(nc, in_maps, core_ids=list(range(N_CORES)),
                               trace=trace)
    y = np.concatenate([r["y"] for r in res.results], axis=0)
    return y.reshape(B, C, H, W), res


def kernel(x: np.ndarray) -> np.ndarray:
    y, _ = _run(np.asarray(x))
    return y


# revision 5
# speedup vs baseline: 1.2978x; 1.1783x over previous
"""2D orthonormal DCT-II over [32,64,224,224], data-parallel on 8 TRN2 cores.

Math per image X [224,224]:  Y = Dh @ X @ Dw.T  (Dh = Dw = 224-pt DCT-II).
Implemented as two PE matmul stages with the *data* as the stationary
operand, which absorbs both transposes:
  stage 1:  T[w,k] = sum_h X[h,w] * DhT[h,k]      (T = (Dh @ X)^T)
  stage 2:  Y[k,l] = sum_w T[w,k] * DwT[w,l]
Matmuls run in float32r (rounded fp32, ~1.6e-4 rel err, 1 cyc/row on the
PE vs 4 for plain fp32). Contraction dim 224 is split 128+96 across two
accumulating matmuls; output partitions 224 likewise split 128+96.
"""
import numpy as np
import concourse.bacc as bacc
import concourse.mybir as mybir
import concourse.tile as tile
from concourse.bass_utils import run_bass_kernel_spmd

B, C, H, W = 32, 64, 224, 224
N_CORES = 8
IMGS = B * C // N_CORES  # images per core
G = 8                    # images per DMA group
P0, P1 = 128, H - 128    # partition split of the 224 dim
NS = 320                 # matmul stream width: 224 real + zero pad (HAM duty)

f32 = mybir.dt.float32
f32r = mybir.dt.float32r

_cache = {}


def _dct2_matrix(n: int) -> np.ndarray:
    k = np.arange(n)[:, None].astype(np.float64)
    m = np.arange(n)[None, :].astype(np.float64)
    d = np.cos(np.pi * (2.0 * m + 1.0) * k / (2.0 * n))
    scale = np.full((n, 1), np.sqrt(2.0 / n))
    scale[0, 0] = np.sqrt(1.0 / n)
    return (scale * d).astype(np.float32)


def _build():
    nc = bacc.Bacc("TRN2", target_bir_lowering=False, debug=False)
    x_d = nc.dram_tensor("x", [IMGS, H, W], f32, kind="ExternalInput").ap()
    dht_d = nc.dram_tensor("dht", [H, H], f32, kind="ExternalInput").ap()
    dwt_d = nc.dram_tensor("dwt", [W, W], f32, kind="ExternalInput").ap()
    y_d = nc.dram_tensor("y", [IMGS, H, W], f32, kind="ExternalOutput").ap()

    with tile.TileContext(nc) as tc:
        with (
            tc.tile_pool(name="consts", bufs=1) as cpool,
            tc.tile_pool(name="xin", bufs=2) as xpool,
            tc.tile_pool(name="xr", bufs=2) as xrpool,
            tc.tile_pool(name="tmid", bufs=4) as tpool,
            tc.tile_pool(name="yout", bufs=2) as ypool,
            tc.tile_pool(name="pst", bufs=2, space="PSUM") as pst,
            tc.tile_pool(name="psy", bufs=2, space="PSUM") as psy,
        ):
            # DCT matrices: stage fp32 (zero-padded to NS cols), round to f32r
            dht_s0 = cpool.tile([P0, NS], f32)
            dht_s1 = cpool.tile([P1, NS], f32)
            dwt_s0 = cpool.tile([P0, NS], f32)
            dwt_s1 = cpool.tile([P1, NS], f32)
            for t in (dht_s0, dht_s1, dwt_s0, dwt_s1):
                nc.gpsimd.memset(t, 0)
            nc.sync.dma_start(dht_s0[:, 0:H], dht_d[0:P0, :])
            nc.sync.dma_start(dht_s1[:, 0:H], dht_d[P0:H, :])
            nc.sync.dma_start(dwt_s0[:, 0:W], dwt_d[0:P0, :])
            nc.sync.dma_start(dwt_s1[:, 0:W], dwt_d[P0:W, :])
            dht0 = cpool.tile([P0, NS], f32r)
            dht1 = cpool.tile([P1, NS], f32r)
            dwt0 = cpool.tile([P0, NS], f32r)
            dwt1 = cpool.tile([P1, NS], f32r)
            nc.vector.tensor_copy(dht0, dht_s0)
            nc.vector.tensor_copy(dht1, dht_s1)
            nc.vector.tensor_copy(dwt0, dwt_s0)
            nc.vector.tensor_copy(dwt1, dwt_s1)

            # PE warmup: ~10us of dense junk matmuls to trip the HAM
            # clock-gate to K=8/8 (2.4 GHz) before the real work starts.
            bf16 = mybir.dt.bfloat16
            junk_w = cpool.tile([P0, P0], bf16)
            junk_m = cpool.tile([P0, 512], bf16)
            nc.gpsimd.memset(junk_w, 0)
            nc.gpsimd.memset(junk_m, 0)
            for r in range(40):
                wp = pst.tile([P0, 512], f32, name=f"warm{r}", tag="t0p")
                nc.tensor.matmul(wp, junk_w, junk_m, start=True, stop=True)

            for g in range(IMGS // G):
                sl = slice(g * G, (g + 1) * G)
                x0 = xpool.tile([P0, G, W], f32, name="x0", tag="x0")
                x1 = xpool.tile([P1, G, W], f32, name="x1", tag="x1")
                nc.sync.dma_start(x0, x_d[sl, 0:P0, :].transpose([1, 0, 2]))
                nc.sync.dma_start(x1, x_d[sl, P0:H, :].transpose([1, 0, 2]))
                x0r = xrpool.tile([P0, G, W], f32r, name="x0r", tag="x0r")
                x1r = xrpool.tile([P1, G, W], f32r, name="x1r", tag="x1r")
                nc.vector.tensor_copy(x0r, x0)
                nc.vector.tensor_copy(x1r, x1)
                ys0 = ypool.tile([P0, G, W], f32, name="ys0", tag="ys0")
                ys1 = ypool.tile([P1, G, W], f32, name="ys1", tag="ys1")

                for j in range(G):
                    # stage 1: T = (Dh @ X)^T, two partition chunks
                    t0p = pst.tile([P0, NS], f32, name="t0p", tag="t0p")
                    t1p = pst.tile([P1, NS], f32, name="t1p", tag="t1p")
                    nc.tensor.matmul(t0p, x0r[:, j, 0:P0], dht0,
                                     start=True, stop=False)
                    nc.tensor.matmul(t0p, x1r[:, j, 0:P0], dht1,
                                     start=False, stop=True)
                    nc.tensor.matmul(t1p, x0r[:, j, P0:W], dht0,
                                     start=True, stop=False)
                    nc.tensor.matmul(t1p, x1r[:, j, P0:W], dht1,
                                     start=False, stop=True)
                    t0r = tpool.tile([P0, H], f32r, name="t0r", tag="t0r")
                    t1r = tpool.tile([P1, H], f32r, name="t1r", tag="t1r")
                    nc.vector.tensor_copy(t0r, t0p[:, 0:H])
                    nc.scalar.copy(t1r, t1p[:, 0:H])
                    # stage 2: Y = T^T @ DwT, two partition chunks
                    y0p = psy.tile([P0, NS], f32, name="y0p", tag="y0p")
                    y1p = psy.tile([P1, NS], f32, name="y1p", tag="y1p")
                    nc.tensor.matmul(y0p, t0r[:, 0:P0], dwt0,
                                     start=True, stop=False)
                    nc.tensor.matmul(y0p, t1r[:, 0:P0], dwt1,
                                     start=False, stop=True)
                    nc.tensor.matmul(y1p, t0r[:, P0:H], dwt0,
                                     start=True, stop=False)
                    nc.tensor.matmul(y1p, t1r[:, P0:H], dwt1,
                                     start=False, stop=True)
                    nc.scalar.copy(ys0[:, j, :], y0p[:, 0:W])
                    nc.vector.tensor_copy(ys1[:, j, :], y1p[:, 0:W])

                nc.sync.dma_start(y_d[sl, 0:P0, :].transpose([1, 0, 2]), ys0)
                nc.sync.dma_start(y_d[sl, P0:H, :].transpose([1, 0, 2]), ys1)

    nc.compile()
    return nc


def _run(x: np.ndarray, trace: bool = False):
    """x: [B, C, H, W] fp32. Returns (y, BassKernelResults)."""
    if "nc" not in _cache:
        _cache["nc"] = _build()
    nc = _cache["nc"]
    d = _dct2_matrix(H)
    dt_ = np.ascontiguousarray(d.T)  # DhT[h, k] = Dh[k, h]; Dh == Dw here
    flat = np.ascontiguousarray(x.reshape(B * C, H, W).astype(np.float32))
    in_maps = [
        {"x": flat[i * IMGS:(i + 1) * IMGS], "dht": dt_, "dwt": dt_}
        for i in range(N_CORES)
    ]
    res = run_bass_kernel_spmd(nc, in_maps, core_ids=list(range(N_CORES)),
                               trace=trace)
    y = np.concatenate([r["y"] for r in res.results], axis=0)
    return y.reshape(B, C, H, W), res


def kernel(x: np.ndarray) -> np.ndarray:
    y, _ = _run(np.asarray(x))
    return y


# revision 7
# speedup vs baseline: 1.3989x; 1.0779x over previous
"""2D orthonormal DCT-II over [32,64,224,224], data-parallel on 8 TRN2 cores.

Math per image X [224,224]:  Y = Dh @ X @ Dw.T  (Dh = Dw = 224-pt DCT-II).
Implemented as two PE matmul stages with the *data* as the stationary
operand, which absorbs both transposes:
  stage 1:  T[w,k] = sum_h X[h,w] * DhT[h,k]      (T = (Dh @ X)^T)
  stage 2:  Y[k,l] = sum_w T[w,k] * DwT[w,l]
Matmuls run in float32r (rounded fp32, ~1.6e-4 rel err, 1 cyc/row on the
PE vs 4 for plain fp32). Contraction dim 224 is split 128+96 across two
accumulating matmuls; output partitions 224 likewise split 128+96.
"""
import numpy as np
import concourse.bacc as bacc
import concourse.mybir as mybir
import concourse.tile as tile
from concourse.bass_utils import run_bass_kernel_spmd

B, C, H, W = 32, 64, 224, 224
N_CORES = 8
IMGS = B * C // N_CORES  # images per core
G = 8                    # images per DMA group
P0, P1 = 128, H - 128    # partition split of the 224 dim
NS = 320                 # matmul stream width: 224 real + zero pad (HAM duty)

f32 = mybir.dt.float32
f32r = mybir.dt.float32r

_cache = {}


def _dct2_matrix(n: int) -> np.ndarray:
    k = np.arange(n)[:, None].astype(np.float64)
    m = np.arange(n)[None, :].astype(np.float64)
    d = np.cos(np.pi * (2.0 * m + 1.0) * k / (2.0 * n))
    scale = np.full((n, 1), np.sqrt(2.0 / n))
    scale[0, 0] = np.sqrt(1.0 / n)
    return (scale * d).astype(np.float32)


def _build():
    nc = bacc.Bacc("TRN2", target_bir_lowering=False, debug=False)
    x_d = nc.dram_tensor("x", [IMGS, H, W], f32, kind="ExternalInput").ap()
    dht_d = nc.dram_tensor("dht", [H, H], f32, kind="ExternalInput").ap()
    dwt_d = nc.dram_tensor("dwt", [W, W], f32, kind="ExternalInput").ap()
    y_d = nc.dram_tensor("y", [IMGS, H, W], f32, kind="ExternalOutput").ap()

    with tile.TileContext(nc) as tc:
        with (
            tc.tile_pool(name="consts", bufs=1) as cpool,
            tc.tile_pool(name="xin", bufs=2) as xpool,
            tc.tile_pool(name="xr", bufs=2) as xrpool,
            tc.tile_pool(name="tmid", bufs=4) as tpool,
            tc.tile_pool(name="yout", bufs=2) as ypool,
            tc.tile_pool(name="pst", bufs=2, space="PSUM") as pst,
            tc.tile_pool(name="psy", bufs=2, space="PSUM") as psy,
        ):
            # DCT matrices: stage fp32 (zero-padded to NS cols), round to f32r
            dht_s0 = cpool.tile([P0, NS], f32)
            dht_s1 = cpool.tile([P1, NS], f32)
            dwt_s0 = cpool.tile([P0, NS], f32)
            dwt_s1 = cpool.tile([P1, NS], f32)
            for t in (dht_s0, dht_s1, dwt_s0, dwt_s1):
                nc.gpsimd.memset(t, 0)
            nc.sync.dma_start(dht_s0[:, 0:H], dht_d[0:P0, :])
            nc.sync.dma_start(dht_s1[:, 0:H], dht_d[P0:H, :])
            nc.sync.dma_start(dwt_s0[:, 0:W], dwt_d[0:P0, :])
            nc.sync.dma_start(dwt_s1[:, 0:W], dwt_d[P0:W, :])
            dht0 = cpool.tile([P0, NS], f32r)
            dht1 = cpool.tile([P1, NS], f32r)
            dwt0 = cpool.tile([P0, NS], f32r)
            dwt1 = cpool.tile([P1, NS], f32r)
            nc.vector.tensor_copy(dht0, dht_s0)
            nc.vector.tensor_copy(dht1, dht_s1)
            nc.vector.tensor_copy(dwt0, dwt_s0)
            nc.vector.tensor_copy(dwt1, dwt_s1)

            # PE warmup: ~10us of dense junk matmuls to trip the HAM
            # clock-gate to K=8/8 (2.4 GHz) before the real work starts.
            bf16 = mybir.dt.bfloat16
            junk_w = cpool.tile([P0, P0], bf16)
            junk_m = cpool.tile([P0, 512], bf16)
            nc.gpsimd.memset(junk_w, 0)
            nc.gpsimd.memset(junk_m, 0)
            for r in range(40):
                wp = pst.tile([P0, 512], f32, name=f"warm{r}", tag="t0p")
                nc.tensor.matmul(wp, junk_w, junk_m, start=True, stop=True)

            def load_group(g):
                sl = slice(g * G, (g + 1) * G)
                x0 = xpool.tile([P0, G, W], f32, name="x0", tag="x0")
                x1 = xpool.tile([P1, G, W], f32, name="x1", tag="x1")
                nc.sync.dma_start(x0, x_d[sl, 0:P0, :].transpose([1, 0, 2]))
                nc.sync.dma_start(x1, x_d[sl, P0:H, :].transpose([1, 0, 2]))
                x0r = xrpool.tile([P0, G, W], f32r, name="x0r", tag="x0r")
                x1r = xrpool.tile([P1, G, W], f32r, name="x1r", tag="x1r")
                nc.vector.tensor_copy(x0r, x0)
                nc.vector.tensor_copy(x1r, x1)
                return x0r, x1r

            NG = IMGS // G
            cur = load_group(0)
            for g in range(NG):
                sl = slice(g * G, (g + 1) * G)
                x0r, x1r = cur
                nxt = None
                ys0 = ypool.tile([P0, G, W], f32, name="ys0", tag="ys0")
                ys1 = ypool.tile([P1, G, W], f32, name="ys1", tag="ys1")

                for j in range(G):
                    if j == 1 and g + 1 < NG:
                        # prefetch next group's load+round while PE crunches
                        nxt = load_group(g + 1)
                    # stage 1: T = (Dh @ X)^T, two partition chunks
                    t0p = pst.tile([P0, NS], f32, name="t0p", tag="t0p")
                    t1p = pst.tile([P1, NS], f32, name="t1p", tag="t1p")
                    nc.tensor.matmul(t0p, x0r[:, j, 0:P0], dht0,
                                     start=True, stop=False)
                    nc.tensor.matmul(t0p, x1r[:, j, 0:P0], dht1,
                                     start=False, stop=True)
                    nc.tensor.matmul(t1p, x0r[:, j, P0:W], dht0,
                                     start=True, stop=False)
                    nc.tensor.matmul(t1p, x1r[:, j, P0:W], dht1,
                                     start=False, stop=True)
                    t0r = tpool.tile([P0, H], f32r, name="t0r", tag="t0r")
                    t1r = tpool.tile([P1, H], f32r, name="t1r", tag="t1r")
                    nc.vector.tensor_copy(t0r, t0p[:, 0:H])
                    nc.scalar.copy(t1r, t1p[:, 0:H])
                    # stage 2: Y = T^T @ DwT, two partition chunks
                    y0p = psy.tile([P0, NS], f32, name="y0p", tag="y0p")
                    y1p = psy.tile([P1, NS], f32, name="y1p", tag="y1p")
                    nc.tensor.matmul(y0p, t0r[:, 0:P0], dwt0,
                                     start=True, stop=False)
                    nc.tensor.matmul(y0p, t1r[:, 0:P0], dwt1,
                                     start=False, stop=True)
                    nc.tensor.matmul(y1p, t0r[:, P0:H], dwt0,
                                     start=True, stop=False)
                    nc.tensor.matmul(y1p, t1r[:, P0:H], dwt1,
                                     start=False, stop=True)
                    nc.scalar.copy(ys0[:, j, :], y0p[:, 0:W])
                    nc.vector.tensor_copy(ys1[:, j, :], y1p[:, 0:W])

                nc.sync.dma_start(y_d[sl, 0:P0, :].transpose([1, 0, 2]), ys0)
                nc.sync.dma_start(y_d[sl, P0:H, :].transpose([1, 0, 2]), ys1)
                cur = nxt

    nc.compile()
    return nc


def _run(x: np.ndarray, trace: bool = False):
    """x: [B, C, H, W] fp32. Returns (y, BassKernelResults)."""
    if "nc" not in _cache:
        _cache["nc"] = _build()
    nc = _cache["nc"]
    d = _dct2_matrix(H)
    dt_ = np.ascontiguousarray(d.T)  # DhT[h, k] = Dh[k, h]; Dh == Dw here
    flat = np.ascontiguousarray(x.reshape(B * C, H, W).astype(np.float32))
    in_maps = [
        {"x": flat[i * IMGS:(i + 1) * IMGS], "dht": dt_, "dwt": dt_}
        for i in range(N_CORES)
    ]
    res = run_bass_kernel_spmd(nc, in_maps, core_ids=list(range(N_CORES)),
                               trace=trace)
    y = np.concatenate([r["y"] for r in res.results], axis=0)
    return y.reshape(B, C, H, W), res


def kernel(x: np.ndarray) -> np.ndarray:
    y, _ = _run(np.asarray(x))
    return y


# revision 8
# speedup vs baseline: 1.4084x; 1.0068x over previous
"""2D orthonormal DCT-II over [32,64,224,224], data-parallel on 8 TRN2 cores.

Math per image X [224,224]:  Y = Dh @ X @ Dw.T  (Dh = Dw = 224-pt DCT-II).
Implemented as two PE matmul stages with the *data* as the stationary
operand, which absorbs both transposes:
  stage 1:  T[w,k] = sum_h X[h,w] * DhT[h,k]      (T = (Dh @ X)^T)
  stage 2:  Y[k,l] = sum_w T[w,k] * DwT[w,l]
Matmuls run in float32r (rounded fp32, ~1.6e-4 rel err, 1 cyc/row on the
PE vs 4 for plain fp32). Contraction dim 224 is split 128+96 across two
accumulating matmuls; output partitions 224 likewise split 128+96.
"""
import numpy as np
import concourse.bacc as bacc
import concourse.mybir as mybir
import concourse.tile as tile
from concourse.bass_utils import run_bass_kernel_spmd

B, C, H, W = 32, 64, 224, 224
N_CORES = 8
IMGS = B * C // N_CORES  # images per core
G = 8                    # images per DMA group
P0, P1 = 128, H - 128    # partition split of the 224 dim
NS = 320                 # matmul stream width: 224 real + zero pad (HAM duty)

f32 = mybir.dt.float32
f32r = mybir.dt.float32r

_cache = {}


def _dct2_matrix(n: int) -> np.ndarray:
    k = np.arange(n)[:, None].astype(np.float64)
    m = np.arange(n)[None, :].astype(np.float64)
    d = np.cos(np.pi * (2.0 * m + 1.0) * k / (2.0 * n))
    scale = np.full((n, 1), np.sqrt(2.0 / n))
    scale[0, 0] = np.sqrt(1.0 / n)
    return (scale * d).astype(np.float32)


def _build():
    nc = bacc.Bacc("TRN2", target_bir_lowering=False, debug=False)
    x_d = nc.dram_tensor("x", [IMGS, H, W], f32, kind="ExternalInput").ap()
    dht_d = nc.dram_tensor("dht", [H, H], f32, kind="ExternalInput").ap()
    dwt_d = nc.dram_tensor("dwt", [W, W], f32, kind="ExternalInput").ap()
    y_d = nc.dram_tensor("y", [IMGS, H, W], f32, kind="ExternalOutput").ap()

    with tile.TileContext(nc) as tc:
        with (
            tc.tile_pool(name="consts", bufs=1) as cpool,
            tc.tile_pool(name="xin", bufs=2) as xpool,
            tc.tile_pool(name="xr", bufs=2) as xrpool,
            tc.tile_pool(name="tmid", bufs=4) as tpool,
            tc.tile_pool(name="yout", bufs=2) as ypool,
            tc.tile_pool(name="pst", bufs=2, space="PSUM") as pst,
            tc.tile_pool(name="psy", bufs=2, space="PSUM") as psy,
        ):
            # DCT matrices: stage fp32 (zero-padded to NS cols), round to f32r
            dht_s0 = cpool.tile([P0, NS], f32)
            dht_s1 = cpool.tile([P1, NS], f32)
            dwt_s0 = cpool.tile([P0, NS], f32)
            dwt_s1 = cpool.tile([P1, NS], f32)
            for t in (dht_s0, dht_s1, dwt_s0, dwt_s1):
                nc.gpsimd.memset(t, 0)
            nc.sync.dma_start(dht_s0[:, 0:H], dht_d[0:P0, :])
            nc.sync.dma_start(dht_s1[:, 0:H], dht_d[P0:H, :])
            nc.sync.dma_start(dwt_s0[:, 0:W], dwt_d[0:P0, :])
            nc.sync.dma_start(dwt_s1[:, 0:W], dwt_d[P0:W, :])
            dht0 = cpool.tile([P0, NS], f32r)
            dht1 = cpool.tile([P1, NS], f32r)
            dwt0 = cpool.tile([P0, NS], f32r)
            dwt1 = cpool.tile([P1, NS], f32r)
            nc.vector.tensor_copy(dht0, dht_s0)
            nc.vector.tensor_copy(dht1, dht_s1)
            nc.vector.tensor_copy(dwt0, dwt_s0)
            nc.vector.tensor_copy(dwt1, dwt_s1)

            # PE warmup: ~10us of dense junk matmuls to trip the HAM
            # clock-gate to K=8/8 (2.4 GHz) before the real work starts.
            bf16 = mybir.dt.bfloat16
            junk_w = cpool.tile([P0, P0], bf16)
            junk_m = cpool.tile([P0, 512], bf16)
            nc.gpsimd.memset(junk_w, 0)
            nc.gpsimd.memset(junk_m, 0)
            for r in range(40):
                wp = pst.tile([P0, 512], f32, name=f"warm{r}", tag="t0p")
                nc.tensor.matmul(wp, junk_w, junk_m, start=True, stop=True)

            def load_group(g):
                sl = slice(g * G, (g + 1) * G)
                x0 = xpool.tile([P0, G, W], f32, name="x0", tag="x0")
                x1 = xpool.tile([P1, G, W], f32, name="x1", tag="x1")
                nc.scalar.dma_start(x0, x_d[sl, 0:P0, :].transpose([1, 0, 2]))
                nc.scalar.dma_start(x1, x_d[sl, P0:H, :].transpose([1, 0, 2]))
                x0r = xrpool.tile([P0, G, W], f32r, name="x0r", tag="x0r")
                x1r = xrpool.tile([P1, G, W], f32r, name="x1r", tag="x1r")
                nc.vector.tensor_copy(x0r, x0)
                nc.gpsimd.tensor_copy(x1r, x1)
                return x0r, x1r

            NG = IMGS // G
            cur = load_group(0)
            for g in range(NG):
                sl = slice(g * G, (g + 1) * G)
                x0r, x1r = cur
                nxt = None
                ys0 = ypool.tile([P0, G, W], f32, name="ys0", tag="ys0")
                ys1 = ypool.tile([P1, G, W], f32, name="ys1", tag="ys1")

                for j in range(G):
                    if j == 1 and g + 1 < NG:
                        # prefetch next group's load+round while PE crunches
                        nxt = load_group(g + 1)
                    # stage 1: T = (Dh @ X)^T, two partition chunks
                    t0p = pst.tile([P0, NS], f32, name="t0p", tag="t0p")
                    t1p = pst.tile([P1, NS], f32, name="t1p", tag="t1p")
                    nc.tensor.matmul(t0p, x0r[:, j, 0:P0], dht0,
                                     start=True, stop=False)
                    nc.tensor.matmul(t0p, x1r[:, j, 0:P0], dht1,
                                     start=False, stop=True)
                    nc.tensor.matmul(t1p, x0r[:, j, P0:W], dht0,
                                     start=True, stop=False)
                    nc.tensor.matmul(t1p, x1r[:, j, P0:W], dht1,
                                     start=False, stop=True)
                    t0r = tpool.tile([P0, H], f32r, name="t0r", tag="t0r")
                    t1r = tpool.tile([P1, H], f32r, name="t1r", tag="t1r")
                    nc.vector.tensor_copy(t0r, t0p[:, 0:H])
                    nc.scalar.copy(t1r, t1p[:, 0:H])
                    # stage 2: Y = T^T @ DwT, two partition chunks
                    y0p = psy.tile([P0, NS], f32, name="y0p", tag="y0p")
                    y1p = psy.tile([P1, NS], f32, name="y1p", tag="y1p")
                    nc.tensor.matmul(y0p, t0r[:, 0:P0], dwt0,
                                     start=True, stop=False)
                    nc.tensor.matmul(y0p, t1r[:, 0:P0], dwt1,
                                     start=False, stop=True)
                    nc.tensor.matmul(y1p, t0r[:, P0:H], dwt0,
                                     start=True, stop=False)
                    nc.tensor.matmul(y1p, t1r[:, P0:H], dwt1,
                                     start=False, stop=True)
                    nc.scalar.copy(ys0[:, j, :], y0p[:, 0:W])
                    nc.vector.tensor_copy(ys1[:, j, :], y1p[:, 0:W])

                nc.sync.dma_start(y_d[sl, 0:P0, :].transpose([1, 0, 2]), ys0)
                nc.sync.dma_start(y_d[sl, P0:H, :].transpose([1, 0, 2]), ys1)
                cur = nxt

    nc.compile()
    return nc


def _run(x: np.ndarray, trace: bool = False):
    """x: [B, C, H, W] fp32. Returns (y, BassKernelResults)."""
    if "nc" not in _cache:
        _cache["nc"] = _build()
    nc = _cache["nc"]
    d = _dct2_matrix(H)
    dt_ = np.ascontiguousarray(d.T)  # DhT[h, k] = Dh[k, h]; Dh == Dw here
    flat = np.ascontiguousarray(x.reshape(B * C, H, W).astype(np.float32))
    in_maps = [
        {"x": flat[i * IMGS:(i + 1) * IMGS], "dht": dt_, "dwt": dt_}
        for i in range(N_CORES)
    ]
    res = run_bass_kernel_spmd(nc, in_maps, core_ids=list(range(N_CORES)),
                               trace=trace)
    y = np.concatenate([r["y"] for r in res.results], axis=0)
    return y.reshape(B, C, H, W), res


def kernel(x: np.ndarray) -> np.ndarray:
    y, _ = _run(np.asarray(x))
    return y
